# revision 23
# baseline (speedup 1.0000x reference)
"""Multi-head causal self-attention (B=4, S=2048, E=1024, H=16) on 8 TRN2 cores.

Sharding: core c handles batch b=c//2 and heads h0=(c%2)*8 .. h0+7 for the
attention, then output-feature half (c%2) for the out-projection. Per q-chunk
the pair {2b, 2b+1} AllGathers its bf16 attention activations (all 16 heads'
features for all 512 rows land on both cores); each core then runs the full-E
out-projection for its own 512 W_out columns and DMAs its half of the output
directly -- no reduce, no collective on the output path.

All matmuls run in bf16 (fp32r streams at ~2 cyc/row on HW and disables fast
weight load; bf16 is 1 cyc/row and halves LDWEIGHTS). PSUM accumulates fp32.
Scores are computed transposed (k on partitions, q free) so probs feed PV
directly; V carries a ones column per head so PV emits softmax denominators.
Biases are folded into the PSUM->SBUF copies on the vector engine (per
-partition scalars for q/k, replicated tiles for v and out) -- no bias matmuls.

Scheduling: the kb-loop is software-pipelined (scores emitted 2 ahead, exp 1
ahead of the PV that consumes it) and the exp for both heads of a pair is one
scalar-engine instruction over a 2-bank PSUM span, amortizing ACT's ~352-cycle
fixed cost. Projection work for chunk c+1 and the out-projection for chunk c-1
are emitted as filler quanta inside chunk c's attention so the PE never idles
(keeps the HAM clock gate at K=8/8) while the scalar engine works through exp.
"""

from contextlib import ExitStack

import ml_dtypes
import numpy as np

import concourse.bass as bass
import concourse.mybir as mybir
import concourse.tile as tile
from concourse import bacc
from concourse.bass_utils import run_bass_kernel_spmd

B, S, E, H = 4, 2048, 1024, 16
HD = E // H          # 64
N_CORES = 8
HLOC = H // 2        # 8 heads per core
ELOC = HLOC * HD     # 512 local features
P = 128
CH = 512             # q-chunk size
NCH = S // CH        # 4
TBPC = CH // P       # 4 token blocks per chunk
NEB = E // P         # 8 contraction blocks
PAIRS = HLOC // 2    # 4 head pairs
NTB = S // P         # 16 token blocks
VW = HD + 1          # 65: v columns per head incl. ones column
NPAIR_G = 2 * PAIRS  # 8 global head pairs
PFX = 10             # qc=3 kb-prefix blocks drained into window 2
F32 = mybir.dt.float32
F32R = mybir.dt.float32r
BF16 = mybir.dt.bfloat16
MASK_VAL = -1e9
BF = ml_dtypes.bfloat16
RGROUPS = [[0, 1], [2, 3], [4, 5], [6, 7]]

_CACHE = {}


def _r(ap):
    return ap.bitcast(mybir.dt.float32r)


class _Filler:
    """Emit queued closures spread uniformly over a window's PV iterations.

    `late` closures only start after `late_frac` of the window has passed
    (used for out-proj quanta that must wait on the previous chunk's
    AllGather)."""

    def __init__(self, early, late, total_iters, late_frac=0.45, front=()):
        self.front = list(front)
        self.early = list(early)
        self.late = list(late)
        self.total = max(total_iters, 1)
        self.late_start = int(late_frac * self.total)
        self.i = 0
        self.f_done = 0
        self.e_done = 0
        self.l_done = 0

    def step(self):
        self.i += 1
        if self.f_done < len(self.front):
            self.front[self.f_done]()
            self.f_done += 1
            return
        et = min(len(self.early), (self.i * len(self.early) + self.total - 1) // self.total)
        while self.e_done < et:
            self.early[self.e_done]()
            self.e_done += 1
        if self.i > self.late_start and self.late:
            span = max(self.total - self.late_start, 1)
            lt = min(len(self.late), ((self.i - self.late_start) * len(self.late) + span - 1) // span)
            while self.l_done < lt:
                self.late[self.l_done]()
                self.l_done += 1

    def flush(self):
        while self.f_done < len(self.front):
            self.front[self.f_done]()
            self.f_done += 1
        while self.e_done < len(self.early):
            self.early[self.e_done]()
            self.e_done += 1
        while self.l_done < len(self.late):
            self.late[self.l_done]()
            self.l_done += 1


def _build_nc():
    nc = bacc.Bacc(
        "TRN2", target_bir_lowering=False, debug=False, num_devices=N_CORES
    )
    qT_in = nc.dram_tensor("qT_in", [E, S], BF16, kind="ExternalInput")
    wq_d = nc.dram_tensor("wq", [E, ELOC], BF16, kind="ExternalInput")
    wk_d = nc.dram_tensor("wk", [E, ELOC], BF16, kind="ExternalInput")
    wv_d = nc.dram_tensor("wv", [E, ELOC], BF16, kind="ExternalInput")
    wo_d = nc.dram_tensor("wo", [E, ELOC], BF16, kind="ExternalInput")
    bqk_d = nc.dram_tensor("bqk", [P, 2 * PAIRS], F32, kind="ExternalInput")
    bv_d = nc.dram_tensor("bv_rep", [P, ELOC], F32, kind="ExternalInput")
    bo_d = nc.dram_tensor("bo_rep", [P, ELOC], F32, kind="ExternalInput")
    ones8_d = nc.dram_tensor("ones8", [P, HLOC], BF16, kind="ExternalInput")
    ones64_d = nc.dram_tensor("ones64", [1, HD], BF16, kind="ExternalInput")
    out_d = nc.dram_tensor("out", [S, ELOC], F32, kind="ExternalOutput")

    with tile.TileContext(nc) as tc, ExitStack() as ctx:
        res = ctx.enter_context(tc.tile_pool(name="res", bufs=1))
        work = ctx.enter_context(tc.tile_pool(name="work", bufs=1))
        scps = ctx.enter_context(tc.tile_pool(name="scps", bufs=1, space="PSUM"))
        pvps = ctx.enter_context(tc.tile_pool(name="pvps", bufs=1, space="PSUM"))
        mmps = ctx.enter_context(tc.tile_pool(name="mmps", bufs=1, space="PSUM"))
        dram = ctx.enter_context(tc.tile_pool(name="dram", bufs=1, space="DRAM"))

        # ---------------- inputs to SBUF ----------------
        # small tensors first: the earliest DVE ops (V ones columns, first
        # proj copies) wait on these; queueing them behind megabytes of
        # weights head-of-line blocks the whole vector engine at startup
        bqk_sb = res.tile([P, 2 * PAIRS], F32, name="t", tag="bqk")
        bv_sb = res.tile([P, ELOC], F32, name="t", tag="bv")
        bo_sb = res.tile([P, ELOC], F32, name="t", tag="bo")
        ones8_sb = res.tile([P, HLOC], BF16, name="t", tag="ones8")
        ones64_sb = res.tile([VW, HD], BF16, name="t", tag="ones64")
        nc.sync.dma_start(ones8_sb[:], ones8_d[:])
        nc.sync.dma_start(bqk_sb[:], bqk_d[:])
        nc.sync.dma_start(ones64_sb[HD:VW, :], ones64_d[:])
        nc.sync.dma_start(bv_sb[:], bv_d[:])
        nc.sync.dma_start(bo_sb[:], bo_d[:])

        # scratch for PE warm-up / AG-wait filler matmuls (no consumers)
        scratch = res.tile([P, CH], BF16, name="t", tag="scratch")
        nc.vector.memset(scratch[:], 0.0)

        def emit_dummy_mms(n, width):
            for _ in range(n):
                ds = scps.tile([P, 2 * CH], F32, name="t", tag="sc", bufs=2)
                nc.tensor.matmul(
                    ds[:, 0:width],
                    scratch[:, 0:P],
                    scratch[:, 0:width],
                    start=True,
                    stop=True,
                )

        wq_sb = [res.tile([P, ELOC], BF16, name="t", tag=f"wq{eb}") for eb in range(NEB)]
        wk_sb = [res.tile([P, ELOC], BF16, name="t", tag=f"wk{eb}") for eb in range(NEB)]
        wv_sb = [res.tile([P, ELOC], BF16, name="t", tag=f"wv{eb}") for eb in range(NEB)]
        wo_sb = [res.tile([P, ELOC], BF16, name="t", tag=f"wo{g}") for g in range(NPAIR_G)]
        for eb in range(NEB):
            nc.sync.dma_start(wq_sb[eb][:], wq_d[eb * P : (eb + 1) * P, :])

        # first chunk of pre-transposed Q
        qt_chunks = {}

        def load_qt(c):
            tiles = [
                work.tile([P, CH], BF16, name="t", tag=f"qt{eb}", bufs=2)
                for eb in range(NEB)
            ]
            for eb in range(NEB):
                nc.sync.dma_start(
                    tiles[eb][:], qT_in[eb * P : (eb + 1) * P, c * CH : (c + 1) * CH]
                )
            qt_chunks[c] = tiles

        load_qt(0)
        # wv before wk: the chunk-0 v-projection is on the critical path to
        # the first PV; wo (only needed ~80us in) goes last
        for eb in range(NEB):
            nc.sync.dma_start(wv_sb[eb][:], wv_d[eb * P : (eb + 1) * P, :])
        for eb in range(NEB):
            nc.sync.dma_start(wk_sb[eb][:], wk_d[eb * P : (eb + 1) * P, :])
        for g in range(NPAIR_G):
            nc.sync.dma_start(wo_sb[g][:], wo_d[g * P : (g + 1) * P, :])

        # warm the HAM clock gate while input DMAs land
        emit_dummy_mms(40, 256)

        # persistent qkv: kT/qT transposed [2-head hd, tok]; V natural with a
        # ones column per head so PV emits denominators.
        kT = [res.tile([P, S], BF16, name="t", tag=f"kT{p}") for p in range(PAIRS)]
        qT = [res.tile([P, S], BF16, name="t", tag=f"qT{p}") for p in range(PAIRS)]
        V = [res.tile([P, HLOC * VW], BF16, name="t", tag=f"V{t}") for t in range(NTB)]
        for t in range(NTB):
            vr = V[t][:].rearrange("p (h c) -> p h c", c=VW)
            nc.vector.tensor_copy(
                vr[:, :, HD:VW], ones8_sb[:].rearrange("p (h c) -> p h c", c=1)
            )

        # ---------------- projection / out-projection quanta ----------------
        def _proj_ps(pstag):
            # up-front quanta rotate through the (idle) 2-slot score pool so
            # consecutive groups double-buffer; in-window quanta use "mm"
            if pstag == "sc":
                return scps.tile([P, 2 * CH], F32, name="t", tag="sc", bufs=2)[:, 0:CH]
            return mmps.tile([P, CH], F32, name="t", tag="mm")

        def emit_qk_group(c, p, which, pstag="mm"):
            w_sb = wq_sb if which == 0 else wk_sb
            dst = qT if which == 0 else kT
            ps = _proj_ps(pstag)
            qt = qt_chunks[c]
            for eb in range(NEB):
                nc.tensor.matmul(
                    ps[:],
                    w_sb[eb][:, p * P : (p + 1) * P],
                    qt[eb][:],
                    start=(eb == 0),
                    stop=(eb == NEB - 1),
                )
            nc.vector.tensor_scalar_add(
                dst[p][:, c * CH : (c + 1) * CH],
                ps[:],
                bqk_sb[:, which * PAIRS + p : which * PAIRS + p + 1],
            )

        def emit_v_group(c, tb, pstag="mm"):
            tbg = c * TBPC + tb
            ps = _proj_ps(pstag)
            qt = qt_chunks[c]
            for eb in range(NEB):
                nc.tensor.matmul(
                    ps[:],
                    qt[eb][:, tb * P : (tb + 1) * P],
                    wv_sb[eb][:],
                    start=(eb == 0),
                    stop=(eb == NEB - 1),
                )
            vr = V[tbg][:].rearrange("p (h c) -> p h c", c=VW)
            nc.vector.tensor_add(
                vr[:, :, 0:HD],
                ps[:].rearrange("p (h c) -> p h c", c=HD),
                bv_sb[:].rearrange("p (h c) -> p h c", c=HD),
            )

        def qk_quanta(c, pstag="mm"):
            q = []
            for p in range(PAIRS):
                for which in range(2):
                    q.append(lambda c=c, p=p, w=which: emit_qk_group(c, p, w, pstag))
            return q

        def v_quanta(c, pstag="mm"):
            return [lambda c=c, tb=tb: emit_v_group(c, tb, pstag) for tb in range(TBPC)]

        def proj_quanta(c, pstag="mm"):
            return qk_quanta(c, pstag) + v_quanta(c, pstag)

        ao_tiles = {}

        def emit_oproj_group(qc, ts):
            ao = ao_tiles[qc]
            ps = mmps.tile([P, CH], F32, name="t", tag="mm")
            for g in range(NPAIR_G):
                nc.tensor.matmul(
                    ps[:],
                    ao[g][:, ts * P : (ts + 1) * P],
                    wo_sb[g][:],
                    start=(g == 0),
                    stop=(g == NPAIR_G - 1),
                )
            o_sb = work.tile([P, CH], F32, name="t", tag="osb", bufs=2)
            nc.vector.tensor_add(o_sb[:], ps[:], bo_sb[:])
            nc.sync.dma_start(
                out_d[qc * CH + ts * P : qc * CH + (ts + 1) * P, :], o_sb[:]
            )

        def oproj_quanta(qc):
            return [lambda qc=qc, ts=ts: emit_oproj_group(qc, ts) for ts in range(TBPC)]

        # ---------------- attention ----------------
        def finalize_pair(p, ape, apo, attn_pair, prefix=None):
            for sub, ap_ in ((0, ape), (1, apo)):
                au = work.tile([VW, CH], F32, name="t", tag="au", bufs=2)
                if prefix is not None:
                    nc.vector.tensor_add(au[:], ap_[:], prefix[sub][:])
                else:
                    nc.vector.tensor_copy(au[:], ap_[:])
                rec = work.tile([VW, CH], F32, name="t", tag="rec", bufs=2)
                # full-range call: custom-DVE ops miscompute on base-64
                # partition slices; rows 0-63 are discarded scratch.
                nc.vector.reciprocal_approx_fast(out=rec[0:VW, :], in_=au[0:VW, :])
                # restage the reciprocal row at base partition 0, then fan it
                # out across 64 partitions on gpsimd (keeps the PE out of it)
                rtop = work.tile([1, CH], F32, name="t", tag="rtop", bufs=2)
                nc.vector.tensor_copy(rtop[:], rec[HD:VW, :])
                bcp = work.tile([HD, CH], F32, name="t", tag="bcp", bufs=2)
                nc.gpsimd.partition_broadcast(bcp[:], rtop[:])
                nc.vector.tensor_mul(
                    attn_pair[sub * HD : (sub + 1) * HD, :], au[0:HD, :], bcp[:]
                )

        def attn_pair_kbs(qc, p, kb_lo, kb_hi, ape, apo, filler, after_prologue=None):
            """Software-pipelined scores -> exp -> causal-zero -> PV over
            kb in [kb_lo, kb_hi), accumulating into the ape/apo PSUM pair."""
            sc_tiles = {}
            pr_tiles = {}
            dlo = qc * TBPC  # first diagonal block index

            def emit_S(kb):
                j = kb - dlo
                lo = j * P if j >= 1 else 0
                sc = scps.tile([P, 2 * CH], F32, name="t", tag="sc", bufs=2)
                for sub in range(2):
                    hb = sub * HD
                    nc.tensor.matmul(
                        sc[:, sub * CH + lo : (sub + 1) * CH],
                        kT[p][hb : hb + HD, kb * P : (kb + 1) * P],
                        qT[p][hb : hb + HD, qc * CH + lo : (qc + 1) * CH],
                        start=True,
                        stop=True,
                    )
                sc_tiles[kb] = (sc, lo)

            def emit_E(kb):
                sc, lo = sc_tiles.pop(kb)
                j = kb - dlo
                pr = work.tile([P, 2 * CH], BF16, name="t", tag="pr", bufs=4)
                scr = sc.rearrange("p (s c) -> p s c", c=CH)
                prr = pr[:].rearrange("p (s c) -> p s c", c=CH)
                nc.scalar.activation(
                    prr[:, :, lo:CH],
                    scr[:, :, lo:CH],
                    mybir.ActivationFunctionType.Exp,
                    scale=1.0 / 8.0,
                )
                if j >= 0:
                    # zero the invalid (k > q) triangle of the probs on the
                    # otherwise-idle gpsimd engine, off the DVE critical path
                    nc.gpsimd.affine_select(
                        out=prr[:, :, lo:CH],
                        in_=prr[:, :, lo:CH],
                        compare_op=mybir.AluOpType.is_ge,
                        fill=0.0,
                        base=0,
                        pattern=[[0, 2], [1, CH - lo]],
                        channel_multiplier=-1,
                    )
                pr_tiles[kb] = (pr, lo)

            def emit_P(kb):
                pr, lo = pr_tiles.pop(kb)
                for sub, ap_ in ((0, ape), (1, apo)):
                    h = 2 * p + sub
                    nc.tensor.matmul(
                        ap_[:, lo:CH],
                        V[kb][:, h * VW : (h + 1) * VW],
                        pr[:, sub * CH + lo : sub * CH + CH],
                        start=(kb == kb_lo),
                        stop=(kb == kb_hi - 1),
                        skip_group_check=True,
                    )

            emit_S(kb_lo)
            if kb_hi - kb_lo > 1:
                emit_S(kb_lo + 1)
            emit_E(kb_lo)
            if after_prologue is not None:
                # deferred work (previous pair's finalize) rides the PE
                # shadow of this prologue
                after_prologue()
            for kb in range(kb_lo, kb_hi):
                if kb + 2 < kb_hi:
                    emit_S(kb + 2)
                if kb + 1 < kb_hi:
                    emit_E(kb + 1)
                emit_P(kb)
                filler.step()

        def new_aps():
            ape = pvps.tile([VW, CH], F32, name="t", tag="ap", bufs=3)
            apo = pvps.tile([VW, CH], F32, name="t", tag="ap", bufs=3)
            return ape, apo

        def exchange(qc, attn_tiles):
            # AllGather bf16 attn over the pair
            ag_i = dram.tile([ELOC, CH], BF16, name="t", tag="agi", bufs=2)
            ag_o = dram.tile([2 * ELOC, CH], BF16, name="t", tag="ago", bufs=2)
            for p in range(PAIRS):
                nc.sync.dma_start(ag_i[p * P : (p + 1) * P, :], attn_tiles[p][:])
            nc.gpsimd.collective_compute(
                "AllGather",
                mybir.AluOpType.bypass,
                replica_groups=RGROUPS,
                ins=[ag_i[:].opt()],
                outs=[ag_o[:].opt()],
            )
            ao = [
                work.tile([P, CH], BF16, name="t", tag=f"ao{g}", bufs=2)
                for g in range(NPAIR_G)
            ]
            for g in range(NPAIR_G):
                nc.sync.dma_start(ao[g][:], ag_o[g * P : (g + 1) * P, :])
            ao_tiles[qc] = ao

        def attn_tile(p):
            return work.tile([P, CH], BF16, name="t", tag=f"attn{p}", bufs=2)

        def window_std(qc, filler):
            attn_tiles = []
            pending = None
            for p in range(PAIRS):
                ape, apo = new_aps()
                at = attn_tile(p)
                attn_tiles.append(at)
                ap_fn = (lambda pd=pending: finalize_pair(*pd)) if pending else None
                attn_pair_kbs(qc, p, 0, (qc + 1) * TBPC, ape, apo, filler, ap_fn)
                pending = (p, ape, apo, at)
            finalize_pair(*pending)
            filler.flush()
            exchange(qc, attn_tiles)

        attn3p = {}

        def window2(filler):
            # attention(qc=2), each pair followed by the qc=3 kb[0:PFX)
            # prefix whose PV partials are flushed to SBUF -- this drains
            # most of the last chunk's exp work into this PE-rich window
            attn_tiles = []
            for p in range(PAIRS):
                ape, apo = new_aps()
                at = attn_tile(p)
                attn_tiles.append(at)
                attn_pair_kbs(2, p, 0, 3 * TBPC, ape, apo, filler)
                ape3, apo3 = new_aps()
                attn_pair_kbs(
                    3, p, 0, PFX, ape3, apo3, filler,
                    after_prologue=lambda p=p, a=ape, b=apo, t=at: finalize_pair(p, a, b, t),
                )
                pe_t = work.tile([VW, CH], F32, name="t", tag=f"pfx{p}e")
                po_t = work.tile([VW, CH], F32, name="t", tag=f"pfx{p}o")
                nc.vector.tensor_copy(pe_t[:], ape3[:])
                nc.vector.tensor_copy(po_t[:], apo3[:])
                attn3p[p] = (pe_t, po_t)
            filler.flush()
            exchange(2, attn_tiles)

        def window3(filler):
            attn_tiles = []
            pending = None
            for p in range(PAIRS):
                ape, apo = new_aps()
                at = attn_tile(p)
                attn_tiles.append(at)
                ap_fn = (lambda pd=pending: finalize_pair(*pd)) if pending else None
                attn_pair_kbs(3, p, PFX, NTB, ape, apo, filler, ap_fn)
                pending = (p, ape, apo, at, attn3p[p])
            finalize_pair(*pending)
            filler.flush()
            exchange(3, attn_tiles)

        # ---------------- schedule ----------------
        # minimal chunk-0 projection (pair 0 q/k + all V blocks), rotating
        # through the idle score pool so consecutive groups double-buffer;
        # the other pairs' q/k projections front-fill window 0. Order q, v, k
        # to match the DMA arrival order of wq/wv/wk.
        emit_qk_group(0, 0, 0, "sc")
        for fn in v_quanta(0, "sc"):
            fn()
        emit_qk_group(0, 0, 1, "sc")

        load_qt(1)
        front0 = []
        for p in range(1, PAIRS):
            for w in range(2):
                front0.append(lambda p=p, w=w: emit_qk_group(0, p, w))
        f = _Filler(proj_quanta(1), [], total_iters=PAIRS * TBPC, front=front0)
        window_std(0, f)

        load_qt(2)
        f = _Filler(proj_quanta(2), oproj_quanta(0), total_iters=PAIRS * 2 * TBPC)
        window_std(1, f)

        load_qt(3)
        f = _Filler(
            qk_quanta(3) + v_quanta(3),
            oproj_quanta(1),
            total_iters=PAIRS * (3 * TBPC + PFX),
        )
        window2(f)

        f = _Filler([], oproj_quanta(2), total_iters=PAIRS * (NTB - PFX), late_frac=0.3)
        window3(f)

        # tail: keep the PE busy (and the clock gate warm) while the last
        # AllGather is in flight, then out-project the last chunk
        emit_dummy_mms(80, CH)
        for fn in oproj_quanta(NCH - 1):
            fn()

    nc.compile()
    return nc


def _in_maps(Q, W_packed, b_packed, W_out, b_out):
    maps = []
    for c in range(N_CORES):
        b = c // 2
        c0 = (c % 2) * ELOC
        bqk = np.zeros((P, 2 * PAIRS), np.float32)
        for p in range(PAIRS):
            bqk[:, p] = b_packed[c0 + p * P : c0 + (p + 1) * P]
            bqk[:, PAIRS + p] = b_packed[E + c0 + p * P : E + c0 + (p + 1) * P]
        maps.append(
            {
                "qT_in": np.ascontiguousarray(Q[b].T).astype(BF),
                "wq": np.ascontiguousarray(W_packed[:, c0 : c0 + ELOC]).astype(BF),
                "wk": np.ascontiguousarray(
                    W_packed[:, E + c0 : E + c0 + ELOC]
                ).astype(BF),
                "wv": np.ascontiguousarray(
                    W_packed[:, 2 * E + c0 : 2 * E + c0 + ELOC]
                ).astype(BF),
                "wo": np.ascontiguousarray(W_out[:, c0 : c0 + ELOC]).astype(BF),
                "bqk": bqk,
                "bv_rep": np.ascontiguousarray(
                    np.broadcast_to(b_packed[2 * E + c0 : 2 * E + c0 + ELOC], (P, ELOC))
                ).astype(np.float32),
                "bo_rep": np.ascontiguousarray(
                    np.broadcast_to(b_out[c0 : c0 + ELOC], (P, ELOC))
                ).astype(np.float32),
                "ones8": np.ones((P, HLOC), BF),
                "ones64": np.ones((1, HD), BF),
            }
        )
    return maps


def _unshard(results):
    out = np.empty((B, S, E), np.float32)
    for b in range(B):
        out[b, :, 0:ELOC] = results[2 * b]["out"]
        out[b, :, ELOC:] = results[2 * b + 1]["out"]
    return out


def kernel(Q, W_packed, b_packed, W_out, b_out):
    Q = np.asarray(Q, np.float32)
    W_packed = np.asarray(W_packed, np.float32)
    b_packed = np.asarray(b_packed, np.float32)
    W_out = np.asarray(W_out, np.float32)
    b_out = np.asarray(b_out, np.float32)

    if "nc" not in _CACHE:
        _CACHE["nc"] = _build_nc()
    nc = _CACHE["nc"]

    maps = _in_maps(Q, W_packed, b_packed, W_out, b_out)
    res = run_bass_kernel_spmd(nc, maps, list(range(N_CORES)))
    return _unshard(res.results)


# revision 25
# speedup vs baseline: 1.1806x; 1.1806x over previous
"""Multi-head causal self-attention (B=4, S=2048, E=1024, H=16) on 8 TRN2 cores.

Sharding: core c handles batch b=c//2 and heads h0=(c%2)*8 .. h0+7 for the
attention, then output-feature half (c%2) for the out-projection. Per q-chunk
the pair {2b, 2b+1} AllGathers its bf16 attention activations (all 16 heads'
features for all 512 rows land on both cores); each core then runs the full-E
out-projection for its own 512 W_out columns and DMAs its half of the output
directly -- no reduce, no collective on the output path.

All matmuls run in bf16 (fp32r streams at ~2 cyc/row on HW and disables fast
weight load; bf16 is 1 cyc/row and halves LDWEIGHTS). PSUM accumulates fp32.
Scores are computed transposed (k on partitions, q free) so probs feed PV
directly; V carries a ones column per head so PV emits softmax denominators.
Biases are folded into the PSUM->SBUF copies on the vector engine (per
-partition scalars for q/k, replicated tiles for v and out) -- no bias matmuls.

Scheduling: the kb-loop is software-pipelined (scores emitted 2 ahead, exp 1
ahead of the PV that consumes it) and the exp for both heads of a pair is one
scalar-engine instruction over a 2-bank PSUM span, amortizing ACT's ~352-cycle
fixed cost. Projection work for chunk c+1 and the out-projection for chunk c-1
are emitted as filler quanta inside chunk c's attention so the PE never idles
(keeps the HAM clock gate at K=8/8) while the scalar engine works through exp.
"""

from contextlib import ExitStack

import ml_dtypes
import numpy as np

import concourse.bass as bass
import concourse.mybir as mybir
import concourse.tile as tile
from concourse import bacc
from concourse.bass_utils import run_bass_kernel_spmd

B, S, E, H = 4, 2048, 1024, 16
HD = E // H          # 64
N_CORES = 8
HLOC = H // 2        # 8 heads per core
ELOC = HLOC * HD     # 512 local features
P = 128
CH = 512             # q-chunk size
NCH = S // CH        # 4
TBPC = CH // P       # 4 token blocks per chunk
NEB = E // P         # 8 contraction blocks
PAIRS = HLOC // 2    # 4 head pairs
NTB = S // P         # 16 token blocks
VW = HD + 1          # 65: v columns per head incl. ones column
NPAIR_G = 2 * PAIRS  # 8 global head pairs
PFX = 10             # qc=3 kb-prefix blocks drained into window 2
F32 = mybir.dt.float32
F32R = mybir.dt.float32r
BF16 = mybir.dt.bfloat16
MASK_VAL = -1e9
BF = ml_dtypes.bfloat16
RGROUPS = [[0, 1], [2, 3], [4, 5], [6, 7]]

_CACHE = {}


def _r(ap):
    return ap.bitcast(mybir.dt.float32r)


class _Filler:
    """Emit queued closures spread uniformly over a window's PV iterations.

    `late` closures only start after `late_frac` of the window has passed
    (used for out-proj quanta that must wait on the previous chunk's
    AllGather)."""

    def __init__(self, early, late, total_iters, late_frac=0.45, front=()):
        self.front = list(front)
        self.early = list(early)
        self.late = list(late)
        self.total = max(total_iters, 1)
        self.late_start = int(late_frac * self.total)
        self.i = 0
        self.f_done = 0
        self.e_done = 0
        self.l_done = 0

    def step(self):
        self.i += 1
        if self.f_done < len(self.front):
            self.front[self.f_done]()
            self.f_done += 1
            return
        et = min(len(self.early), (self.i * len(self.early) + self.total - 1) // self.total)
        while self.e_done < et:
            self.early[self.e_done]()
            self.e_done += 1
        if self.i > self.late_start and self.late:
            span = max(self.total - self.late_start, 1)
            lt = min(len(self.late), ((self.i - self.late_start) * len(self.late) + span - 1) // span)
            while self.l_done < lt:
                self.late[self.l_done]()
                self.l_done += 1

    def flush(self):
        while self.f_done < len(self.front):
            self.front[self.f_done]()
            self.f_done += 1
        while self.e_done < len(self.early):
            self.early[self.e_done]()
            self.e_done += 1
        while self.l_done < len(self.late):
            self.late[self.l_done]()
            self.l_done += 1


def _build_nc():
    nc = bacc.Bacc(
        "TRN2", target_bir_lowering=False, debug=False, num_devices=N_CORES
    )
    qT_in = nc.dram_tensor("qT_in", [E, S], BF16, kind="ExternalInput")
    wq_d = nc.dram_tensor("wq", [E, ELOC], BF16, kind="ExternalInput")
    wk_d = nc.dram_tensor("wk", [E, ELOC], BF16, kind="ExternalInput")
    wv_d = nc.dram_tensor("wv", [E, ELOC], BF16, kind="ExternalInput")
    wo_d = nc.dram_tensor("wo", [E, ELOC], BF16, kind="ExternalInput")
    bqk_d = nc.dram_tensor("bqk", [P, 2 * PAIRS], F32, kind="ExternalInput")
    bv_d = nc.dram_tensor("bv_rep", [P, ELOC], F32, kind="ExternalInput")
    bo_d = nc.dram_tensor("bo_rep", [P, ELOC], F32, kind="ExternalInput")
    ones8_d = nc.dram_tensor("ones8", [P, HLOC], BF16, kind="ExternalInput")
    ones64_d = nc.dram_tensor("ones64", [1, HD], BF16, kind="ExternalInput")
    out_d = nc.dram_tensor("out", [S, ELOC], F32, kind="ExternalOutput")

    with tile.TileContext(nc) as tc, ExitStack() as ctx:
        res = ctx.enter_context(tc.tile_pool(name="res", bufs=1))
        work = ctx.enter_context(tc.tile_pool(name="work", bufs=1))
        scps = ctx.enter_context(tc.tile_pool(name="scps", bufs=1, space="PSUM"))
        pvps = ctx.enter_context(tc.tile_pool(name="pvps", bufs=1, space="PSUM"))
        mmps = ctx.enter_context(tc.tile_pool(name="mmps", bufs=1, space="PSUM"))
        dram = ctx.enter_context(tc.tile_pool(name="dram", bufs=1, space="DRAM"))

        # ---------------- inputs to SBUF ----------------
        # small tensors first: the earliest DVE ops (V ones columns, first
        # proj copies) wait on these; queueing them behind megabytes of
        # weights head-of-line blocks the whole vector engine at startup
        bqk_sb = res.tile([P, 2 * PAIRS], F32, name="t", tag="bqk")
        bv_sb = res.tile([P, ELOC], F32, name="t", tag="bv")
        bo_sb = res.tile([P, ELOC], F32, name="t", tag="bo")
        ones8_sb = res.tile([P, HLOC], BF16, name="t", tag="ones8")
        ones64_sb = res.tile([VW, HD], BF16, name="t", tag="ones64")
        nc.sync.dma_start(ones8_sb[:], ones8_d[:])
        nc.sync.dma_start(bqk_sb[:], bqk_d[:])
        nc.sync.dma_start(ones64_sb[HD:VW, :], ones64_d[:])
        nc.sync.dma_start(bv_sb[:], bv_d[:])
        nc.sync.dma_start(bo_sb[:], bo_d[:])

        # scratch for PE warm-up / AG-wait filler matmuls (no consumers)
        scratch = res.tile([P, CH], BF16, name="t", tag="scratch")
        nc.vector.memset(scratch[:], 0.0)

        def emit_dummy_mms(n, width):
            for _ in range(n):
                ds = scps.tile([P, 2 * CH], F32, name="t", tag="sc", bufs=2)
                nc.tensor.matmul(
                    ds[:, 0:width],
                    scratch[:, 0:P],
                    scratch[:, 0:width],
                    start=True,
                    stop=True,
                )

        wq_sb = [res.tile([P, ELOC], BF16, name="t", tag=f"wq{eb}") for eb in range(NEB)]
        wk_sb = [res.tile([P, ELOC], BF16, name="t", tag=f"wk{eb}") for eb in range(NEB)]
        wv_sb = [res.tile([P, ELOC], BF16, name="t", tag=f"wv{eb}") for eb in range(NEB)]
        wo_sb = [res.tile([P, ELOC], BF16, name="t", tag=f"wo{g}") for g in range(NPAIR_G)]
        for eb in range(NEB):
            nc.sync.dma_start(wq_sb[eb][:], wq_d[eb * P : (eb + 1) * P, :])

        # first chunk of pre-transposed Q
        qt_chunks = {}

        def load_qt(c):
            tiles = [
                work.tile([P, CH], BF16, name="t", tag=f"qt{eb}", bufs=2)
                for eb in range(NEB)
            ]
            for eb in range(NEB):
                nc.sync.dma_start(
                    tiles[eb][:], qT_in[eb * P : (eb + 1) * P, c * CH : (c + 1) * CH]
                )
            qt_chunks[c] = tiles

        load_qt(0)
        # wv before wk: the chunk-0 v-projection is on the critical path to
        # the first PV; wo (only needed ~80us in) goes last
        for eb in range(NEB):
            nc.sync.dma_start(wv_sb[eb][:], wv_d[eb * P : (eb + 1) * P, :])
        for eb in range(NEB):
            nc.sync.dma_start(wk_sb[eb][:], wk_d[eb * P : (eb + 1) * P, :])
        for g in range(NPAIR_G):
            nc.sync.dma_start(wo_sb[g][:], wo_d[g * P : (g + 1) * P, :])

        # warm the HAM clock gate while input DMAs land
        emit_dummy_mms(40, 256)

        # persistent qkv: kT/qT transposed [2-head hd, tok]; V natural with a
        # ones column per head so PV emits denominators.
        kT = [res.tile([P, S], BF16, name="t", tag=f"kT{p}") for p in range(PAIRS)]
        qT = [res.tile([P, S], BF16, name="t", tag=f"qT{p}") for p in range(PAIRS)]
        V = [res.tile([P, HLOC * VW], BF16, name="t", tag=f"V{t}") for t in range(NTB)]
        for t in range(NTB):
            vr = V[t][:].rearrange("p (h c) -> p h c", c=VW)
            nc.vector.tensor_copy(
                vr[:, :, HD:VW], ones8_sb[:].rearrange("p (h c) -> p h c", c=1)
            )

        # ---------------- projection / out-projection quanta ----------------
        def _proj_ps(pstag):
            # up-front quanta rotate through the (idle) 2-slot score pool so
            # consecutive groups double-buffer; in-window quanta use "mm"
            if pstag == "sc":
                return scps.tile([P, 2 * CH], F32, name="t", tag="sc", bufs=2)[:, 0:CH]
            return mmps.tile([P, CH], F32, name="t", tag="mm")

        def emit_qk_group(c, p, which, pstag="mm"):
            w_sb = wq_sb if which == 0 else wk_sb
            dst = qT if which == 0 else kT
            ps = _proj_ps(pstag)
            qt = qt_chunks[c]
            for eb in range(NEB):
                nc.tensor.matmul(
                    ps[:],
                    w_sb[eb][:, p * P : (p + 1) * P],
                    qt[eb][:],
                    start=(eb == 0),
                    stop=(eb == NEB - 1),
                )
            nc.vector.tensor_scalar_add(
                dst[p][:, c * CH : (c + 1) * CH],
                ps[:],
                bqk_sb[:, which * PAIRS + p : which * PAIRS + p + 1],
            )

        def emit_v_group(c, tb, pstag="mm"):
            tbg = c * TBPC + tb
            ps = _proj_ps(pstag)
            qt = qt_chunks[c]
            for eb in range(NEB):
                nc.tensor.matmul(
                    ps[:],
                    qt[eb][:, tb * P : (tb + 1) * P],
                    wv_sb[eb][:],
                    start=(eb == 0),
                    stop=(eb == NEB - 1),
                )
            vr = V[tbg][:].rearrange("p (h c) -> p h c", c=VW)
            nc.vector.tensor_add(
                vr[:, :, 0:HD],
                ps[:].rearrange("p (h c) -> p h c", c=HD),
                bv_sb[:].rearrange("p (h c) -> p h c", c=HD),
            )

        def qk_quanta(c, pstag="mm"):
            q = []
            for p in range(PAIRS):
                for which in range(2):
                    q.append(lambda c=c, p=p, w=which: emit_qk_group(c, p, w, pstag))
            return q

        def v_quanta(c, pstag="mm"):
            return [lambda c=c, tb=tb: emit_v_group(c, tb, pstag) for tb in range(TBPC)]

        def proj_quanta(c, pstag="mm"):
            return qk_quanta(c, pstag) + v_quanta(c, pstag)

        ao_tiles = {}

        def emit_oproj_group(qc, ts):
            ao = ao_tiles[qc]
            ps = mmps.tile([P, CH], F32, name="t", tag="mm")
            for g in range(NPAIR_G):
                nc.tensor.matmul(
                    ps[:],
                    ao[g][:, ts * P : (ts + 1) * P],
                    wo_sb[g][:],
                    start=(g == 0),
                    stop=(g == NPAIR_G - 1),
                )
            o_sb = work.tile([P, CH], F32, name="t", tag="osb", bufs=2)
            nc.vector.tensor_add(o_sb[:], ps[:], bo_sb[:])
            nc.sync.dma_start(
                out_d[qc * CH + ts * P : qc * CH + (ts + 1) * P, :], o_sb[:]
            )

        def oproj_quanta(qc):
            return [lambda qc=qc, ts=ts: emit_oproj_group(qc, ts) for ts in range(TBPC)]

        # ---------------- attention ----------------
        def finalize_pair(p, ape, apo, attn_pair, prefix=None):
            for sub, ap_ in ((0, ape), (1, apo)):
                au = work.tile([VW, CH], F32, name="t", tag="au", bufs=2)
                if prefix is not None:
                    nc.vector.tensor_add(au[:], ap_[:], prefix[sub][:])
                else:
                    nc.vector.tensor_copy(au[:], ap_[:])
                rec = work.tile([VW, CH], F32, name="t", tag="rec", bufs=2)
                # full-range call: custom-DVE ops miscompute on base-64
                # partition slices; rows 0-63 are discarded scratch.
                nc.vector.reciprocal_approx_fast(out=rec[0:VW, :], in_=au[0:VW, :])
                rec_bf = work.tile([VW, CH], BF16, name="t", tag="recbf", bufs=2)
                nc.vector.tensor_copy(rec_bf[HD:VW, :], rec[HD:VW, :])
                bcp = mmps.tile([HD, CH], F32, name="t", tag="mm")
                nc.tensor.matmul(
                    bcp[:],
                    ones64_sb[HD:VW, :],
                    rec_bf[HD:VW, :],
                    start=True,
                    stop=True,
                )
                nc.vector.tensor_mul(
                    attn_pair[sub * HD : (sub + 1) * HD, :], au[0:HD, :], bcp[:]
                )

        def attn_pair_kbs(qc, p, kb_lo, kb_hi, ape, apo, filler, after_prologue=None):
            """Software-pipelined scores -> exp -> causal-zero -> PV over
            kb in [kb_lo, kb_hi), accumulating into the ape/apo PSUM pair."""
            sc_tiles = {}
            pr_tiles = {}
            dlo = qc * TBPC  # first diagonal block index

            def emit_S(kb):
                j = kb - dlo
                lo = j * P if j >= 1 else 0
                sc = scps.tile([P, 2 * CH], F32, name="t", tag="sc", bufs=2)
                for sub in range(2):
                    hb = sub * HD
                    nc.tensor.matmul(
                        sc[:, sub * CH + lo : (sub + 1) * CH],
                        kT[p][hb : hb + HD, kb * P : (kb + 1) * P],
                        qT[p][hb : hb + HD, qc * CH + lo : (qc + 1) * CH],
                        start=True,
                        stop=True,
                    )
                sc_tiles[kb] = (sc, lo)

            def emit_E(kb):
                sc, lo = sc_tiles.pop(kb)
                j = kb - dlo
                pr = work.tile([P, 2 * CH], BF16, name="t", tag="pr", bufs=4)
                scr = sc.rearrange("p (s c) -> p s c", c=CH)
                prr = pr[:].rearrange("p (s c) -> p s c", c=CH)
                nc.scalar.activation(
                    prr[:, :, lo:CH],
                    scr[:, :, lo:CH],
                    mybir.ActivationFunctionType.Exp,
                    scale=1.0 / 8.0,
                )
                if j >= 0:
                    # zero the invalid (k > q) triangle of the probs on the
                    # otherwise-idle gpsimd engine, off the DVE critical path
                    nc.gpsimd.affine_select(
                        out=prr[:, :, lo:CH],
                        in_=prr[:, :, lo:CH],
                        compare_op=mybir.AluOpType.is_ge,
                        fill=0.0,
                        base=0,
                        pattern=[[0, 2], [1, CH - lo]],
                        channel_multiplier=-1,
                    )
                pr_tiles[kb] = (pr, lo)

            def emit_P(kb):
                pr, lo = pr_tiles.pop(kb)
                for sub, ap_ in ((0, ape), (1, apo)):
                    h = 2 * p + sub
                    nc.tensor.matmul(
                        ap_[:, lo:CH],
                        V[kb][:, h * VW : (h + 1) * VW],
                        pr[:, sub * CH + lo : sub * CH + CH],
                        start=(kb == kb_lo),
                        stop=(kb == kb_hi - 1),
                        skip_group_check=True,
                    )

            emit_S(kb_lo)
            if kb_hi - kb_lo > 1:
                emit_S(kb_lo + 1)
            emit_E(kb_lo)
            if after_prologue is not None:
                # deferred work (previous pair's finalize) rides the PE
                # shadow of this prologue
                after_prologue()
            for kb in range(kb_lo, kb_hi):
                if kb + 2 < kb_hi:
                    emit_S(kb + 2)
                if kb + 1 < kb_hi:
                    emit_E(kb + 1)
                emit_P(kb)
                filler.step()

        def new_aps():
            ape = pvps.tile([VW, CH], F32, name="t", tag="ap", bufs=3)
            apo = pvps.tile([VW, CH], F32, name="t", tag="ap", bufs=3)
            return ape, apo

        def exchange(qc, attn_tiles):
            # AllGather bf16 attn over the pair
            ag_i = dram.tile([ELOC, CH], BF16, name="t", tag="agi", bufs=2)
            ag_o = dram.tile([2 * ELOC, CH], BF16, name="t", tag="ago", bufs=2)
            for p in range(PAIRS):
                nc.sync.dma_start(ag_i[p * P : (p + 1) * P, :], attn_tiles[p][:])
            nc.gpsimd.collective_compute(
                "AllGather",
                mybir.AluOpType.bypass,
                replica_groups=RGROUPS,
                ins=[ag_i[:].opt()],
                outs=[ag_o[:].opt()],
            )
            ao = [
                work.tile([P, CH], BF16, name="t", tag=f"ao{g}", bufs=2)
                for g in range(NPAIR_G)
            ]
            for g in range(NPAIR_G):
                nc.sync.dma_start(ao[g][:], ag_o[g * P : (g + 1) * P, :])
            ao_tiles[qc] = ao

        def attn_tile(p):
            return work.tile([P, CH], BF16, name="t", tag=f"attn{p}", bufs=2)

        def window_std(qc, filler):
            attn_tiles = []
            pending = None
            for p in range(PAIRS):
                ape, apo = new_aps()
                at = attn_tile(p)
                attn_tiles.append(at)
                ap_fn = (lambda pd=pending: finalize_pair(*pd)) if pending else None
                attn_pair_kbs(qc, p, 0, (qc + 1) * TBPC, ape, apo, filler, ap_fn)
                pending = (p, ape, apo, at)
            finalize_pair(*pending)
            filler.flush()
            exchange(qc, attn_tiles)

        attn3p = {}

        def window2(filler):
            # attention(qc=2), each pair followed by the qc=3 kb[0:PFX)
            # prefix whose PV partials are flushed to SBUF -- this drains
            # most of the last chunk's exp work into this PE-rich window
            attn_tiles = []
            for p in range(PAIRS):
                ape, apo = new_aps()
                at = attn_tile(p)
                attn_tiles.append(at)
                attn_pair_kbs(2, p, 0, 3 * TBPC, ape, apo, filler)
                ape3, apo3 = new_aps()
                attn_pair_kbs(
                    3, p, 0, PFX, ape3, apo3, filler,
                    after_prologue=lambda p=p, a=ape, b=apo, t=at: finalize_pair(p, a, b, t),
                )
                pe_t = work.tile([VW, CH], F32, name="t", tag=f"pfx{p}e")
                po_t = work.tile([VW, CH], F32, name="t", tag=f"pfx{p}o")
                nc.vector.tensor_copy(pe_t[:], ape3[:])
                nc.vector.tensor_copy(po_t[:], apo3[:])
                attn3p[p] = (pe_t, po_t)
            filler.flush()
            exchange(2, attn_tiles)

        def window3(filler):
            attn_tiles = []
            pending = None
            for p in range(PAIRS):
                ape, apo = new_aps()
                at = attn_tile(p)
                attn_tiles.append(at)
                ap_fn = (lambda pd=pending: finalize_pair(*pd)) if pending else None
                attn_pair_kbs(3, p, PFX, NTB, ape, apo, filler, ap_fn)
                pending = (p, ape, apo, at, attn3p[p])
            finalize_pair(*pending)
            filler.flush()
            exchange(3, attn_tiles)

        # ---------------- schedule ----------------
        # minimal chunk-0 projection (pair 0 q/k + all V blocks), rotating
        # through the idle score pool so consecutive groups double-buffer;
        # the other pairs' q/k projections front-fill window 0. Order q, v, k
        # to match the DMA arrival order of wq/wv/wk.
        emit_qk_group(0, 0, 0, "sc")
        for fn in v_quanta(0, "sc"):
            fn()
        emit_qk_group(0, 0, 1, "sc")

        load_qt(1)
        front0 = []
        for p in range(1, PAIRS):
            for w in range(2):
                front0.append(lambda p=p, w=w: emit_qk_group(0, p, w))
        f = _Filler(proj_quanta(1), [], total_iters=PAIRS * TBPC, front=front0)
        window_std(0, f)

        load_qt(2)
        load_qt(3)
        # chunk-3 q/k projection rides window 1 (window 2 is the fullest)
        f = _Filler(
            proj_quanta(2) + qk_quanta(3),
            oproj_quanta(0),
            total_iters=PAIRS * 2 * TBPC,
        )
        window_std(1, f)

        f = _Filler(
            v_quanta(3), oproj_quanta(1), total_iters=PAIRS * (3 * TBPC + PFX)
        )
        window2(f)

        f = _Filler([], oproj_quanta(2), total_iters=PAIRS * (NTB - PFX), late_frac=0.3)
        window3(f)

        # tail: keep the PE busy (and the clock gate warm) while the last
        # AllGather is in flight, then out-project the last chunk
        emit_dummy_mms(80, CH)
        for fn in oproj_quanta(NCH - 1):
            fn()

    nc.compile()
    return nc


def _in_maps(Q, W_packed, b_packed, W_out, b_out):
    maps = []
    for c in range(N_CORES):
        b = c // 2
        c0 = (c % 2) * ELOC
        bqk = np.zeros((P, 2 * PAIRS), np.float32)
        for p in range(PAIRS):
            bqk[:, p] = b_packed[c0 + p * P : c0 + (p + 1) * P]
            bqk[:, PAIRS + p] = b_packed[E + c0 + p * P : E + c0 + (p + 1) * P]
        maps.append(
            {
                "qT_in": np.ascontiguousarray(Q[b].T).astype(BF),
                "wq": np.ascontiguousarray(W_packed[:, c0 : c0 + ELOC]).astype(BF),
                "wk": np.ascontiguousarray(
                    W_packed[:, E + c0 : E + c0 + ELOC]
                ).astype(BF),
                "wv": np.ascontiguousarray(
                    W_packed[:, 2 * E + c0 : 2 * E + c0 + ELOC]
                ).astype(BF),
                "wo": np.ascontiguousarray(W_out[:, c0 : c0 + ELOC]).astype(BF),
                "bqk": bqk,
                "bv_rep": np.ascontiguousarray(
                    np.broadcast_to(b_packed[2 * E + c0 : 2 * E + c0 + ELOC], (P, ELOC))
                ).astype(np.float32),
                "bo_rep": np.ascontiguousarray(
                    np.broadcast_to(b_out[c0 : c0 + ELOC], (P, ELOC))
                ).astype(np.float32),
                "ones8": np.ones((P, HLOC), BF),
                "ones64": np.ones((1, HD), BF),
            }
        )
    return maps


def _unshard(results):
    out = np.empty((B, S, E), np.float32)
    for b in range(B):
        out[b, :, 0:ELOC] = results[2 * b]["out"]
        out[b, :, ELOC:] = results[2 * b + 1]["out"]
    return out


def kernel(Q, W_packed, b_packed, W_out, b_out):
    Q = np.asarray(Q, np.float32)
    W_packed = np.asarray(W_packed, np.float32)
    b_packed = np.asarray(b_packed, np.float32)
    W_out = np.asarray(W_out, np.float32)
    b_out = np.asarray(b_out, np.float32)

    if "nc" not in _CACHE:
        _CACHE["nc"] = _build_nc()
    nc = _CACHE["nc"]

    maps = _in_maps(Q, W_packed, b_packed, W_out, b_out)
    res = run_bass_kernel_spmd(nc, maps, list(range(N_CORES)))
    return _unshard(res.results)


# revision 31
# speedup vs baseline: 1.1930x; 1.0105x over previous
"""Multi-head causal self-attention (B=4, S=2048, E=1024, H=16) on 8 TRN2 cores.

Sharding: core c handles batch b=c//2 and heads h0=(c%2)*8 .. h0+7 for the
attention, then output-feature half (c%2) for the out-projection. Per q-chunk
the pair {2b, 2b+1} AllGathers its bf16 attention activations (all 16 heads'
features for all 512 rows land on both cores); each core then runs the full-E
out-projection for its own 512 W_out columns and DMAs its half of the output
directly -- no reduce, no collective on the output path.

All matmuls run in bf16 (fp32r streams at ~2 cyc/row on HW and disables fast
weight load; bf16 is 1 cyc/row and halves LDWEIGHTS). PSUM accumulates fp32.
Scores are computed transposed (k on partitions, q free) so probs feed PV
directly; V carries a ones column per head so PV emits softmax denominators.
Biases are folded into the PSUM->SBUF copies on the vector engine (per
-partition scalars for q/k, replicated tiles for v and out) -- no bias matmuls.

Scheduling: the kb-loop is software-pipelined (scores emitted 2 ahead, exp 1
ahead of the PV that consumes it) and the exp for both heads of a pair is one
scalar-engine instruction over a 2-bank PSUM span, amortizing ACT's ~352-cycle
fixed cost. Projection work for chunk c+1 and the out-projection for chunk c-1
are emitted as filler quanta inside chunk c's attention so the PE never idles
(keeps the HAM clock gate at K=8/8) while the scalar engine works through exp.
"""

from contextlib import ExitStack

import ml_dtypes
import numpy as np

import concourse.bass as bass
import concourse.mybir as mybir
import concourse.tile as tile
from concourse import bacc
from concourse.bass_utils import run_bass_kernel_spmd

B, S, E, H = 4, 2048, 1024, 16
HD = E // H          # 64
N_CORES = 8
HLOC = H // 2        # 8 heads per core
ELOC = HLOC * HD     # 512 local features
P = 128
CH = 512             # q-chunk size
NCH = S // CH        # 4
TBPC = CH // P       # 4 token blocks per chunk
NEB = E // P         # 8 contraction blocks
PAIRS = HLOC // 2    # 4 head pairs
NTB = S // P         # 16 token blocks
VW = HD + 1          # 65: v columns per head incl. ones column
NPAIR_G = 2 * PAIRS  # 8 global head pairs
PFX = 10             # qc=3 kb-prefix blocks drained into window 2
F32 = mybir.dt.float32
F32R = mybir.dt.float32r
BF16 = mybir.dt.bfloat16
MASK_VAL = -1e9
BF = ml_dtypes.bfloat16
RGROUPS = [[0, 1], [2, 3], [4, 5], [6, 7]]

_CACHE = {}


def _r(ap):
    return ap.bitcast(mybir.dt.float32r)


class _Filler:
    """Emit queued closures spread uniformly over a window's PV iterations.

    `late` closures only start after `late_frac` of the window has passed
    (used for out-proj quanta that must wait on the previous chunk's
    AllGather)."""

    def __init__(self, early, late, total_iters, late_frac=0.45, front=()):
        self.front = list(front)
        self.early = list(early)
        self.late = list(late)
        self.total = max(total_iters, 1)
        self.late_start = int(late_frac * self.total)
        self.i = 0
        self.f_done = 0
        self.e_done = 0
        self.l_done = 0

    def step(self):
        self.i += 1
        if self.f_done < len(self.front):
            self.front[self.f_done]()
            self.f_done += 1
            return
        et = min(len(self.early), (self.i * len(self.early) + self.total - 1) // self.total)
        while self.e_done < et:
            self.early[self.e_done]()
            self.e_done += 1
        if self.i > self.late_start and self.late:
            span = max(self.total - self.late_start, 1)
            lt = min(len(self.late), ((self.i - self.late_start) * len(self.late) + span - 1) // span)
            while self.l_done < lt:
                self.late[self.l_done]()
                self.l_done += 1

    def flush(self):
        while self.f_done < len(self.front):
            self.front[self.f_done]()
            self.f_done += 1
        while self.e_done < len(self.early):
            self.early[self.e_done]()
            self.e_done += 1
        while self.l_done < len(self.late):
            self.late[self.l_done]()
            self.l_done += 1


def _build_nc():
    nc = bacc.Bacc(
        "TRN2", target_bir_lowering=False, debug=False, num_devices=N_CORES
    )
    qT_in = nc.dram_tensor("qT_in", [E, S], BF16, kind="ExternalInput")
    wq_d = nc.dram_tensor("wq", [E, ELOC], BF16, kind="ExternalInput")
    wk_d = nc.dram_tensor("wk", [E, ELOC], BF16, kind="ExternalInput")
    wv_d = nc.dram_tensor("wv", [E, ELOC], BF16, kind="ExternalInput")
    wo_d = nc.dram_tensor("wo", [E, ELOC], BF16, kind="ExternalInput")
    bqk_d = nc.dram_tensor("bqk", [P, 2 * PAIRS], F32, kind="ExternalInput")
    bv_d = nc.dram_tensor("bv_rep", [P, ELOC], F32, kind="ExternalInput")
    bo_d = nc.dram_tensor("bo_rep", [P, ELOC], F32, kind="ExternalInput")
    ones8_d = nc.dram_tensor("ones8", [P, HLOC], BF16, kind="ExternalInput")
    ones64_d = nc.dram_tensor("ones64", [1, HD], BF16, kind="ExternalInput")
    out_d = nc.dram_tensor("out", [S, ELOC], F32, kind="ExternalOutput")

    with tile.TileContext(nc) as tc, ExitStack() as ctx:
        res = ctx.enter_context(tc.tile_pool(name="res", bufs=1))
        work = ctx.enter_context(tc.tile_pool(name="work", bufs=1))
        scps = ctx.enter_context(tc.tile_pool(name="scps", bufs=1, space="PSUM"))
        pvps = ctx.enter_context(tc.tile_pool(name="pvps", bufs=1, space="PSUM"))
        mmps = ctx.enter_context(tc.tile_pool(name="mmps", bufs=1, space="PSUM"))
        dram = ctx.enter_context(tc.tile_pool(name="dram", bufs=1, space="DRAM"))

        # ---------------- inputs to SBUF ----------------
        # small tensors first: the earliest DVE ops (V ones columns, first
        # proj copies) wait on these; queueing them behind megabytes of
        # weights head-of-line blocks the whole vector engine at startup
        bqk_sb = res.tile([P, 2 * PAIRS], F32, name="t", tag="bqk")
        bv_sb = res.tile([P, ELOC], F32, name="t", tag="bv")
        bo_sb = res.tile([P, ELOC], F32, name="t", tag="bo")
        ones8_sb = res.tile([P, HLOC], BF16, name="t", tag="ones8")
        ones64_sb = res.tile([VW, HD], BF16, name="t", tag="ones64")
        nc.sync.dma_start(ones8_sb[:], ones8_d[:])
        nc.sync.dma_start(bqk_sb[:], bqk_d[:])
        nc.sync.dma_start(ones64_sb[HD:VW, :], ones64_d[:])
        nc.sync.dma_start(bv_sb[:], bv_d[:])
        nc.sync.dma_start(bo_sb[:], bo_d[:])

        # scratch for PE warm-up / AG-wait filler matmuls (no consumers)
        scratch = res.tile([P, CH], BF16, name="t", tag="scratch")
        nc.vector.memset(scratch[:], 0.0)

        def emit_dummy_mms(n, width):
            for _ in range(n):
                ds = scps.tile([P, 2 * CH], F32, name="t", tag="sc", bufs=2)
                nc.tensor.matmul(
                    ds[:, 0:width],
                    scratch[:, 0:P],
                    scratch[:, 0:width],
                    start=True,
                    stop=True,
                )

        wq_sb = [res.tile([P, ELOC], BF16, name="t", tag=f"wq{eb}") for eb in range(NEB)]
        wk_sb = [res.tile([P, ELOC], BF16, name="t", tag=f"wk{eb}") for eb in range(NEB)]
        wv_sb = [res.tile([P, ELOC], BF16, name="t", tag=f"wv{eb}") for eb in range(NEB)]
        wo_sb = [res.tile([P, ELOC], BF16, name="t", tag=f"wo{g}") for g in range(NPAIR_G)]

        # first chunk of pre-transposed Q
        qt_chunks = {}

        def load_qt(c):
            tiles = [
                work.tile([P, CH], BF16, name="t", tag=f"qt{eb}", bufs=2)
                for eb in range(NEB)
            ]
            for eb in range(NEB):
                nc.sync.dma_start(
                    tiles[eb][:], qT_in[eb * P : (eb + 1) * P, c * CH : (c + 1) * CH]
                )
            qt_chunks[c] = tiles

        # DMA order tracks the critical path to the first exp: pair-0 slices
        # of wq/wk + chunk-0 Q first, then wv, then the rest
        for eb in range(NEB):
            nc.sync.dma_start(wq_sb[eb][:, 0:P], wq_d[eb * P : (eb + 1) * P, 0:P])
        load_qt(0)
        for eb in range(NEB):
            nc.sync.dma_start(wk_sb[eb][:, 0:P], wk_d[eb * P : (eb + 1) * P, 0:P])
        for eb in range(NEB):
            nc.sync.dma_start(wv_sb[eb][:], wv_d[eb * P : (eb + 1) * P, :])
        for eb in range(NEB):
            nc.sync.dma_start(
                wq_sb[eb][:, P:ELOC], wq_d[eb * P : (eb + 1) * P, P:ELOC]
            )
        for eb in range(NEB):
            nc.sync.dma_start(
                wk_sb[eb][:, P:ELOC], wk_d[eb * P : (eb + 1) * P, P:ELOC]
            )
        for g in range(NPAIR_G):
            nc.sync.dma_start(wo_sb[g][:], wo_d[g * P : (g + 1) * P, :])

        # warm the HAM clock gate while input DMAs land
        emit_dummy_mms(40, 256)

        # persistent qkv: kT/qT transposed [2-head hd, tok]; V natural with a
        # ones column per head so PV emits denominators.
        kT = [res.tile([P, S], BF16, name="t", tag=f"kT{p}") for p in range(PAIRS)]
        qT = [res.tile([P, S], BF16, name="t", tag=f"qT{p}") for p in range(PAIRS)]
        V = [res.tile([P, HLOC * VW], BF16, name="t", tag=f"V{t}") for t in range(NTB)]
        for t in range(NTB):
            vr = V[t][:].rearrange("p (h c) -> p h c", c=VW)
            nc.vector.tensor_copy(
                vr[:, :, HD:VW], ones8_sb[:].rearrange("p (h c) -> p h c", c=1)
            )

        # ---------------- projection / out-projection quanta ----------------
        def _proj_ps(pstag):
            # up-front quanta rotate through the (idle) 2-slot score pool so
            # consecutive groups double-buffer; in-window quanta use "mm"
            if pstag == "sc":
                return scps.tile([P, 2 * CH], F32, name="t", tag="sc", bufs=2)[:, 0:CH]
            return mmps.tile([P, CH], F32, name="t", tag="mm")

        def emit_qk_group(c, p, which, pstag="mm"):
            w_sb = wq_sb if which == 0 else wk_sb
            dst = qT if which == 0 else kT
            ps = _proj_ps(pstag)
            qt = qt_chunks[c]
            for eb in range(NEB):
                nc.tensor.matmul(
                    ps[:],
                    w_sb[eb][:, p * P : (p + 1) * P],
                    qt[eb][:],
                    start=(eb == 0),
                    stop=(eb == NEB - 1),
                )
            nc.vector.tensor_scalar_add(
                dst[p][:, c * CH : (c + 1) * CH],
                ps[:],
                bqk_sb[:, which * PAIRS + p : which * PAIRS + p + 1],
            )

        def emit_v_group(c, tb, pstag="mm"):
            tbg = c * TBPC + tb
            ps = _proj_ps(pstag)
            qt = qt_chunks[c]
            for eb in range(NEB):
                nc.tensor.matmul(
                    ps[:],
                    qt[eb][:, tb * P : (tb + 1) * P],
                    wv_sb[eb][:],
                    start=(eb == 0),
                    stop=(eb == NEB - 1),
                )
            vr = V[tbg][:].rearrange("p (h c) -> p h c", c=VW)
            nc.vector.tensor_add(
                vr[:, :, 0:HD],
                ps[:].rearrange("p (h c) -> p h c", c=HD),
                bv_sb[:].rearrange("p (h c) -> p h c", c=HD),
            )

        def qk_quanta(c, pstag="mm"):
            q = []
            for p in range(PAIRS):
                for which in range(2):
                    q.append(lambda c=c, p=p, w=which: emit_qk_group(c, p, w, pstag))
            return q

        def v_quanta(c, pstag="mm"):
            return [lambda c=c, tb=tb: emit_v_group(c, tb, pstag) for tb in range(TBPC)]

        def proj_quanta(c, pstag="mm"):
            return qk_quanta(c, pstag) + v_quanta(c, pstag)

        ao_tiles = {}

        def emit_oproj_group(qc, ts, pstag="mm"):
            ao = ao_tiles[qc]
            ps = _proj_ps(pstag)
            for g in range(NPAIR_G):
                nc.tensor.matmul(
                    ps[:],
                    ao[g][:, ts * P : (ts + 1) * P],
                    wo_sb[g][:],
                    start=(g == 0),
                    stop=(g == NPAIR_G - 1),
                )
            o_sb = work.tile([P, CH], F32, name="t", tag="osb", bufs=2)
            nc.vector.tensor_add(o_sb[:], ps[:], bo_sb[:])
            nc.sync.dma_start(
                out_d[qc * CH + ts * P : qc * CH + (ts + 1) * P, :], o_sb[:]
            )

        def oproj_quanta(qc, alternate=False):
            return [
                lambda qc=qc, ts=ts: emit_oproj_group(
                    qc, ts, "sc" if alternate and ts % 2 else "mm"
                )
                for ts in range(TBPC)
            ]

        # ---------------- attention ----------------
        def finalize_pair(p, ape, apo, attn_pair, prefix=None):
            for sub, ap_ in ((0, ape), (1, apo)):
                au = work.tile([VW, CH], F32, name="t", tag="au", bufs=2)
                if prefix is not None:
                    nc.vector.tensor_add(au[:], ap_[:], prefix[sub][:])
                else:
                    nc.vector.tensor_copy(au[:], ap_[:])
                rec = work.tile([VW, CH], F32, name="t", tag="rec", bufs=2)
                # full-range call: custom-DVE ops miscompute on base-64
                # partition slices; rows 0-63 are discarded scratch.
                nc.vector.reciprocal_approx_fast(out=rec[0:VW, :], in_=au[0:VW, :])
                rec_bf = work.tile([VW, CH], BF16, name="t", tag="recbf", bufs=2)
                nc.vector.tensor_copy(rec_bf[HD:VW, :], rec[HD:VW, :])
                bcp = mmps.tile([HD, CH], F32, name="t", tag="mm")
                nc.tensor.matmul(
                    bcp[:],
                    ones64_sb[HD:VW, :],
                    rec_bf[HD:VW, :],
                    start=True,
                    stop=True,
                )
                nc.vector.tensor_mul(
                    attn_pair[sub * HD : (sub + 1) * HD, :], au[0:HD, :], bcp[:]
                )

        def attn_pair_kbs(qc, p, kb_lo, kb_hi, ape, apo, filler, after_prologue=None):
            """Software-pipelined scores -> exp -> causal-zero -> PV over
            kb in [kb_lo, kb_hi), accumulating into the ape/apo PSUM pair."""
            sc_tiles = {}
            pr_tiles = {}
            dlo = qc * TBPC  # first diagonal block index

            def emit_S(kb):
                j = kb - dlo
                lo = j * P if j >= 1 else 0
                sc = scps.tile([P, 2 * CH], F32, name="t", tag="sc", bufs=2)
                for sub in range(2):
                    hb = sub * HD
                    nc.tensor.matmul(
                        sc[:, sub * CH + lo : (sub + 1) * CH],
                        kT[p][hb : hb + HD, kb * P : (kb + 1) * P],
                        qT[p][hb : hb + HD, qc * CH + lo : (qc + 1) * CH],
                        start=True,
                        stop=True,
                    )
                sc_tiles[kb] = (sc, lo)

            def emit_E(kb):
                sc, lo = sc_tiles.pop(kb)
                j = kb - dlo
                pr = work.tile([P, 2 * CH], BF16, name="t", tag="pr", bufs=4)
                scr = sc.rearrange("p (s c) -> p s c", c=CH)
                prr = pr[:].rearrange("p (s c) -> p s c", c=CH)
                nc.scalar.activation(
                    prr[:, :, lo:CH],
                    scr[:, :, lo:CH],
                    mybir.ActivationFunctionType.Exp,
                    scale=1.0 / 8.0,
                )
                if j >= 0:
                    # zero the invalid (k > q) triangle of the probs on the
                    # otherwise-idle gpsimd engine, off the DVE critical path
                    nc.gpsimd.affine_select(
                        out=prr[:, :, lo:CH],
                        in_=prr[:, :, lo:CH],
                        compare_op=mybir.AluOpType.is_ge,
                        fill=0.0,
                        base=0,
                        pattern=[[0, 2], [1, CH - lo]],
                        channel_multiplier=-1,
                    )
                pr_tiles[kb] = (pr, lo)

            def emit_P(kb):
                pr, lo = pr_tiles.pop(kb)
                for sub, ap_ in ((0, ape), (1, apo)):
                    h = 2 * p + sub
                    nc.tensor.matmul(
                        ap_[:, lo:CH],
                        V[kb][:, h * VW : (h + 1) * VW],
                        pr[:, sub * CH + lo : sub * CH + CH],
                        start=(kb == kb_lo),
                        stop=(kb == kb_hi - 1),
                        skip_group_check=True,
                    )

            emit_S(kb_lo)
            if kb_hi - kb_lo > 1:
                emit_S(kb_lo + 1)
            emit_E(kb_lo)
            if after_prologue is not None:
                # deferred work (previous pair's finalize) rides the PE
                # shadow of this prologue
                after_prologue()
            for kb in range(kb_lo, kb_hi):
                if kb + 2 < kb_hi:
                    emit_S(kb + 2)
                if kb + 1 < kb_hi:
                    emit_E(kb + 1)
                emit_P(kb)
                filler.step()

        def new_aps():
            ape = pvps.tile([VW, CH], F32, name="t", tag="ap", bufs=3)
            apo = pvps.tile([VW, CH], F32, name="t", tag="ap", bufs=3)
            return ape, apo

        def exchange(qc, attn_tiles):
            # AllGather bf16 attn over the pair
            ag_i = dram.tile([ELOC, CH], BF16, name="t", tag="agi", bufs=2)
            ag_o = dram.tile([2 * ELOC, CH], BF16, name="t", tag="ago", bufs=2)
            for p in range(PAIRS):
                nc.sync.dma_start(ag_i[p * P : (p + 1) * P, :], attn_tiles[p][:])
            nc.gpsimd.collective_compute(
                "AllGather",
                mybir.AluOpType.bypass,
                replica_groups=RGROUPS,
                ins=[ag_i[:].opt()],
                outs=[ag_o[:].opt()],
            )
            ao = [
                work.tile([P, CH], BF16, name="t", tag=f"ao{g}", bufs=2)
                for g in range(NPAIR_G)
            ]
            for g in range(NPAIR_G):
                nc.sync.dma_start(ao[g][:], ag_o[g * P : (g + 1) * P, :])
            ao_tiles[qc] = ao

        def attn_tile(p):
            return work.tile([P, CH], BF16, name="t", tag=f"attn{p}", bufs=2)

        def window_std(qc, filler):
            attn_tiles = []
            pending = None
            for p in range(PAIRS):
                ape, apo = new_aps()
                at = attn_tile(p)
                attn_tiles.append(at)
                ap_fn = (lambda pd=pending: finalize_pair(*pd)) if pending else None
                attn_pair_kbs(qc, p, 0, (qc + 1) * TBPC, ape, apo, filler, ap_fn)
                pending = (p, ape, apo, at)
            finalize_pair(*pending)
            filler.flush()
            exchange(qc, attn_tiles)

        attn3p = {}

        def window2(filler):
            # attention(qc=2), each pair followed by the qc=3 kb[0:PFX)
            # prefix whose PV partials are flushed to SBUF -- this drains
            # most of the last chunk's exp work into this PE-rich window
            attn_tiles = []
            for p in range(PAIRS):
                ape, apo = new_aps()
                at = attn_tile(p)
                attn_tiles.append(at)
                attn_pair_kbs(2, p, 0, 3 * TBPC, ape, apo, filler)
                ape3, apo3 = new_aps()
                attn_pair_kbs(
                    3, p, 0, PFX, ape3, apo3, filler,
                    after_prologue=lambda p=p, a=ape, b=apo, t=at: finalize_pair(p, a, b, t),
                )
                pe_t = work.tile([VW, CH], F32, name="t", tag=f"pfx{p}e")
                po_t = work.tile([VW, CH], F32, name="t", tag=f"pfx{p}o")
                nc.vector.tensor_copy(pe_t[:], ape3[:])
                nc.vector.tensor_copy(po_t[:], apo3[:])
                attn3p[p] = (pe_t, po_t)
            filler.flush()
            exchange(2, attn_tiles)

        def exchange_pair(qc, p, at, ao_map):
            # per-pair AllGather: pairs 0-2 exchange while later pairs still
            # compute, so only pair 3's 128KB gather sits in the tail
            ag_i = dram.tile([P, CH], BF16, name="t", tag="agip", bufs=2)
            ag_o = dram.tile([2 * P, CH], BF16, name="t", tag="agop", bufs=2)
            nc.sync.dma_start(ag_i[:], at[:])
            nc.gpsimd.collective_compute(
                "AllGather",
                mybir.AluOpType.bypass,
                replica_groups=RGROUPS,
                ins=[ag_i[:].opt()],
                outs=[ag_o[:].opt()],
            )
            for k_, g in ((0, p), (1, p + PAIRS)):
                t = work.tile([P, CH], BF16, name="t", tag=f"ao{g}", bufs=2)
                nc.sync.dma_start(t[:], ag_o[k_ * P : (k_ + 1) * P, :])
                ao_map[g] = t

        def window3(filler):
            pending = None
            ao_map = {}

            def fin_ex(pd):
                finalize_pair(*pd)
                exchange_pair(3, pd[0], pd[3], ao_map)

            for p in range(PAIRS):
                ape, apo = new_aps()
                at = attn_tile(p)
                ap_fn = (lambda pd=pending: fin_ex(pd)) if pending else None
                attn_pair_kbs(3, p, PFX, NTB, ape, apo, filler, ap_fn)
                pending = (p, ape, apo, at, attn3p[p])
            fin_ex(pending)
            filler.flush()
            ao_tiles[3] = [ao_map[g] for g in range(NPAIR_G)]

        # ---------------- schedule ----------------
        # minimal chunk-0 projection (pair 0 q/k + all V blocks), rotating
        # through the idle score pool so consecutive groups double-buffer;
        # the other pairs' q/k projections front-fill window 0. Order q, k, v
        # to match the DMA arrival order (pair-0 wq/wk slices land first).
        emit_qk_group(0, 0, 0, "sc")
        emit_qk_group(0, 0, 1, "sc")
        for fn in v_quanta(0, "sc"):
            fn()

        load_qt(1)
        front0 = []
        for p in range(1, PAIRS):
            for w in range(2):
                front0.append(lambda p=p, w=w: emit_qk_group(0, p, w))
        f = _Filler(proj_quanta(1), [], total_iters=PAIRS * TBPC, front=front0)
        window_std(0, f)

        load_qt(2)
        load_qt(3)
        # chunk-3 q/k projection rides window 1 (window 2 is the fullest)
        f = _Filler(
            proj_quanta(2) + qk_quanta(3),
            oproj_quanta(0),
            total_iters=PAIRS * 2 * TBPC,
        )
        window_std(1, f)

        f = _Filler(
            v_quanta(3), oproj_quanta(1), total_iters=PAIRS * (3 * TBPC + PFX)
        )
        window2(f)

        f = _Filler([], oproj_quanta(2), total_iters=PAIRS * (NTB - PFX), late_frac=0.3)
        window3(f)

        # tail: keep the PE busy (and the clock gate warm) while the last
        # pair's AllGather is in flight, then out-project the last chunk
        emit_dummy_mms(40, CH)
        for fn in oproj_quanta(NCH - 1, alternate=True):
            fn()

    nc.compile()
    return nc


def _in_maps(Q, W_packed, b_packed, W_out, b_out):
    maps = []
    for c in range(N_CORES):
        b = c // 2
        c0 = (c % 2) * ELOC
        bqk = np.zeros((P, 2 * PAIRS), np.float32)
        for p in range(PAIRS):
            bqk[:, p] = b_packed[c0 + p * P : c0 + (p + 1) * P]
            bqk[:, PAIRS + p] = b_packed[E + c0 + p * P : E + c0 + (p + 1) * P]
        maps.append(
            {
                "qT_in": np.ascontiguousarray(Q[b].T).astype(BF),
                "wq": np.ascontiguousarray(W_packed[:, c0 : c0 + ELOC]).astype(BF),
                "wk": np.ascontiguousarray(
                    W_packed[:, E + c0 : E + c0 + ELOC]
                ).astype(BF),
                "wv": np.ascontiguousarray(
                    W_packed[:, 2 * E + c0 : 2 * E + c0 + ELOC]
                ).astype(BF),
                "wo": np.ascontiguousarray(W_out[:, c0 : c0 + ELOC]).astype(BF),
                "bqk": bqk,
                "bv_rep": np.ascontiguousarray(
                    np.broadcast_to(b_packed[2 * E + c0 : 2 * E + c0 + ELOC], (P, ELOC))
                ).astype(np.float32),
                "bo_rep": np.ascontiguousarray(
                    np.broadcast_to(b_out[c0 : c0 + ELOC], (P, ELOC))
                ).astype(np.float32),
                "ones8": np.ones((P, HLOC), BF),
                "ones64": np.ones((1, HD), BF),
            }
        )
    return maps


def _unshard(results):
    out = np.empty((B, S, E), np.float32)
    for b in range(B):
        out[b, :, 0:ELOC] = results[2 * b]["out"]
        out[b, :, ELOC:] = results[2 * b + 1]["out"]
    return out


def kernel(Q, W_packed, b_packed, W_out, b_out):
    Q = np.asarray(Q, np.float32)
    W_packed = np.asarray(W_packed, np.float32)
    b_packed = np.asarray(b_packed, np.float32)
    W_out = np.asarray(W_out, np.float32)
    b_out = np.asarray(b_out, np.float32)

    if "nc" not in _CACHE:
        _CACHE["nc"] = _build_nc()
    nc = _CACHE["nc"]

    maps = _in_maps(Q, W_packed, b_packed, W_out, b_out)
    res = run_bass_kernel_spmd(nc, maps, list(range(N_CORES)))
    return _unshard(res.results)
